# revision 1
# baseline (speedup 1.0000x reference)
"""Trainium2 Bass kernel for a dense transformer encoder block (B=4, S=2048,
D=1024, H=16, MLP=4096).

Sharding: 8 cores = 4 batch elements x 2 query-halves, no collectives. Each
core's kv sequence is host-reordered so its 1024 query tokens come first
(attention is permutation-invariant over keys), so Q/residual tensors are
plain slices of the kv set. K/V are computed for the full 2048-token sequence
(~6% duplicated FLOPs vs. perfect sharding).

Per-core dataflow is feature-major ("T" = [feature, token]) so every matmul
has contraction dim 128 on partitions (sub-128-contraction matmuls fail to
load on this stack, all dtypes):
  LN1 (token-major, bn_stats) -> PE-transpose -> xnT            [phase 1]
  per head-group of 4 heads: Q/K/V projections from xnT         [phase 3]
    scores^T = KT_pair^T @ Qpad   (zero-padded rhs selects one head
                                   of the packed pair; K=128 kept)
    exp on ACT, scale=1/8 fused, both heads in one [128,1024] op -> f32r
    AV+den fused: lhsT = [V_head | 1 | 0] so psum rows 0:64 = V^T e and
      row 64 = sum(e); one augmented matmul per (ktile, head)
    reciprocal of row 64, partition-broadcast via DRAM round-trip DMA
      (stride-0 partition APs are DRAM-only), multiply -> RT; head B's
      rows shift 0:64 -> 64:128 via a small SBUF->SBUF DMA
  O-proj +bo, PE-transpose back, +residual -> x2 -> DRAM        [phase 4a]
  LN2 on x2 -> PE-transpose -> xn2T                             [phase 4b]
  MLP: h1 (+b1 and exact-erf Gelu fused on ACT), h2 (+b2),      [phase 5]
    PE-transpose back, +x2 residual -> out

Numerics: matmuls in float32r (TF32-class, ~1.5e-4 rel err, full PE rate at
free-dim >= 256; requires producers typed f32r), fp32 PSUM accumulation,
fp32 layernorm/softmax scalars. End-to-end rel err ~1.4e-4.

Weights are host-retiled to [tile, partition, kd, m] so each weight-tile DMA
is one contiguous block (4KB per-partition chunks). LN affine (g=1, b=0 for
this problem's inputs) is skipped at build time when the host detects
identity values; a full-affine variant is built otherwise.

Cost-model (TimelineSim) span: ~862 us/core; PE busy ~820 us (the binding
engine; attention runs at 50% array utilization, the price of the K=128
constraint with DH=64 heads and no working sub-128 row/col tiling).
"""

import os
import sys

sys.path.insert(0, "/opt/trn_rl_repo")

from contextlib import ExitStack

import numpy as np

import concourse.bass as bass
import concourse.tile as tile
from concourse import bacc, bass_utils, mybir
from concourse.masks import make_identity

F32 = mybir.dt.float32
F32R = mybir.dt.float32r
BF16 = mybir.dt.bfloat16
AF = mybir.ActivationFunctionType
ALU = mybir.AluOpType

B, S, D = 4, 2048, 1024
H, DH, MLP = 16, 64, 4096
P = 128
KD = D // P            # 8 partition tiles over D
FT = MLP // P          # 32 partition tiles over MLP dim
NQ = S // 2            # 1024 query tokens per core
ST = S // P            # 16 kv token tiles
QTT = NQ // P          # 8 q token tiles
QS = 512               # free-dim slice
NQS = NQ // QS         # 2
NKS = S // QS          # 4
NG = 4                 # head groups
EPS = 1e-6
DEBUG = bool(int(os.environ.get("KERNEL_DEBUG", "0")))
MLP_BF16 = bool(int(os.environ.get("KERNEL_MLP_BF16", "0")))

_CACHE = {}


def _build(ln_affine=True, mlp_bf16=True):
    nc = bacc.Bacc(None, target_bir_lowering=False, debug=False, num_devices=8)

    xkv = nc.dram_tensor("xkv", [S, D], F32, kind="ExternalInput").ap()
    # weights arrive host-tiled: [tile, p, kd, m] so each SBUF weight tile is
    # one contiguous DRAM block (4KB+ per-partition DMA chunks)
    Wq = nc.dram_tensor("Wq", [KD, P, KD, P], F32R, kind="ExternalInput").ap()
    Wk = nc.dram_tensor("Wk", [KD, P, KD, P], F32R, kind="ExternalInput").ap()
    Wv = nc.dram_tensor("Wv", [NG, P, KD, 256], F32R, kind="ExternalInput").ap()
    Wo = nc.dram_tensor("Wo", [KD, P, KD, P], F32R, kind="ExternalInput").ap()
    W1 = nc.dram_tensor("W1", [FT, P, KD, P], F32R, kind="ExternalInput").ap()
    W2 = nc.dram_tensor("W2", [KD, P, FT, P], BF16 if mlp_bf16 else F32R, kind="ExternalInput").ap()
    bq = nc.dram_tensor("bq", [D], F32, kind="ExternalInput").ap()
    bk = nc.dram_tensor("bk", [D], F32, kind="ExternalInput").ap()
    bv = nc.dram_tensor("bv", [D], F32, kind="ExternalInput").ap()
    bo = nc.dram_tensor("bo", [D], F32, kind="ExternalInput").ap()
    b1 = nc.dram_tensor("b1", [MLP], F32, kind="ExternalInput").ap()
    b2 = nc.dram_tensor("b2", [D], F32, kind="ExternalInput").ap()
    g1 = nc.dram_tensor("g1", [D], F32, kind="ExternalInput").ap()
    be1 = nc.dram_tensor("be1", [D], F32, kind="ExternalInput").ap()
    g2 = nc.dram_tensor("g2", [D], F32, kind="ExternalInput").ap()
    be2 = nc.dram_tensor("be2", [D], F32, kind="ExternalInput").ap()
    out = nc.dram_tensor("out", [NQ, D], F32, kind="ExternalOutput").ap()

    dbg = {}
    if DEBUG:
        dbg["xnkvT"] = nc.dram_tensor("d_xnkvT", [P, KD, S], F32R, kind="ExternalOutput").ap()
        dbg["qt0"] = nc.dram_tensor("d_qt0", [P, 2, NQ], F32R, kind="ExternalOutput").ap()
        dbg["kt0"] = nc.dram_tensor("d_kt0", [P, 2, S], F32R, kind="ExternalOutput").ap()
        dbg["v0"] = nc.dram_tensor("d_v0", [P, ST, 2, 2, P], F32R, kind="ExternalOutput").ap()
        dbg["rt"] = nc.dram_tensor("d_rt", [P, KD, NQ], F32R, kind="ExternalOutput").ap()
        dbg["e0"] = nc.dram_tensor("d_e0", [P, QS], F32R, kind="ExternalOutput").ap()
        dbg["s0"] = nc.dram_tensor("d_s0", [P, QS], F32, kind="ExternalOutput").ap()
        dbg["av0"] = nc.dram_tensor("d_av0", [65, 2, QS], F32, kind="ExternalOutput").ap()
        dbg["x2"] = nc.dram_tensor("d_x2", [P, QTT, D], F32, kind="ExternalOutput").ap()

    def bcast_ap(vec):
        # [D] dram vector -> [128, D] partition-replicated DMA source
        return bass.AP(tensor=vec.tensor, offset=vec.offset, ap=[[0, P]] + list(vec.ap))



    with tile.TileContext(nc) as tc:
        es = ExitStack()
        params = es.enter_context(tc.tile_pool(name="params", bufs=1))
        dramp = es.enter_context(tc.tile_pool(name="dram", bufs=1, space="DRAM"))
        x2d = dramp.tile([P, QTT, D], F32)

        ident_f = params.tile([P, P], F32)
        make_identity(nc, ident_f)
        ident = params.tile([P, P], F32R)
        nc.vector.tensor_copy(ident[:], ident_f[:])
        ones_f = params.tile([P, 1], F32)
        nc.vector.memset(ones_f[:, 0:1], 1.0)

        def pvec(v, n, nm):  # [n*128] -> [128, n] (dim o*128+p -> [p, o])
            t = params.tile([P, n], F32, name=nm)
            nc.sync.dma_start(t[:], v.rearrange("(o p) -> p o", p=P))
            return t

        bq_t = pvec(bq, KD, "bq_t")
        bk_t = pvec(bk, KD, "bk_t")
        bo_t = pvec(bo, KD, "bo_t")
        b2_t = pvec(b2, KD, "b2_t")
        b1_t = pvec(b1, FT, "b1_t")
        bv_rep = params.tile([P, D], F32)
        nc.gpsimd.dma_start(bv_rep[:], bcast_ap(bv))

        rt_es = ExitStack()
        rtp = rt_es.enter_context(tc.tile_pool(name="rt", bufs=1))
        RT_h = [rtp.tile([P, KD, QS], F32R, name=f"RT{h}") for h in range(NQS)]

        xn_es = ExitStack()
        xnp = xn_es.enter_context(tc.tile_pool(name="xn", bufs=1))
        xn_kvT = xnp.tile([P, KD, S], F32R)

        # ---- Phase 1: LN1 + transpose to feature-major ----
        with tc.tile_pool(name="p1tmp", bufs=4) as p1t, \
             tc.tile_pool(name="p1s", bufs=4) as p1s, \
             tc.tile_pool(name="ln1", bufs=1) as ln1p, \
             tc.tile_pool(name="p1ps", bufs=6, space="PSUM") as ps1:
            g1_rep = ln1p.tile([P, D], F32)
            nc.gpsimd.dma_start(g1_rep[:], bcast_ap(g1))
            be1_rep = ln1p.tile([P, D], F32)
            nc.gpsimd.dma_start(be1_rep[:], bcast_ap(be1))
            eps_t = ln1p.tile([P, 1], F32)
            nc.vector.memset(eps_t[:], EPS)

            for t in range(ST):
                x_t = p1t.tile([P, D], F32, tag="x_t")
                nc.sync.dma_start(x_t[:], xkv[t * P:(t + 1) * P, :])
                stats = p1s.tile([P, 2, 6], F32, tag="stats")
                xv = x_t[:].rearrange("p (s f) -> p s f", s=2)
                for s in range(2):
                    nc.vector.bn_stats(stats[:, s, :], xv[:, s, :])
                mv = p1s.tile([P, 2], F32, tag="mv")
                nc.vector.bn_aggr(mv[:], stats[:])
                std = p1s.tile([P, 1], F32, tag="std")
                nc.scalar.activation(std[:], mv[:, 1:2], AF.Sqrt, bias=eps_t[:])
                nc.vector.reciprocal(std[:], std[:])
                xn_t = p1t.tile([P, D], F32R, tag="xn_t")
                nc.vector.tensor_scalar(
                    xn_t[:], x_t[:], scalar1=mv[:, 0:1], scalar2=std[:],
                    op0=ALU.subtract, op1=ALU.mult)
                if ln_affine:
                    nc.vector.tensor_tensor(xn_t[:], xn_t[:], g1_rep[:], ALU.mult)
                    nc.vector.tensor_tensor(xn_t[:], xn_t[:], be1_rep[:], ALU.add)
                for j2 in range(KD // 2):
                    pst = ps1.tile([P, 2, P], F32, tag="tp")
                    for h in range(2):
                        nc.tensor.transpose(
                            pst[:, h, :].bitcast(F32R),
                            xn_t[:, (2 * j2 + h) * P:(2 * j2 + h + 1) * P], ident[:])
                    nc.vector.tensor_copy(
                        xn_kvT[:, 2 * j2:2 * j2 + 2, t * P:(t + 1) * P], pst[:])

        if DEBUG:
            nc.sync.dma_start(dbg["xnkvT"], xn_kvT[:])

        # ---- Phase 3: per-group QKV projection + attention ----
        with tc.tile_pool(name="kv", bufs=1) as kvp, \
             tc.tile_pool(name="wst", bufs=2) as wsp, \
             tc.tile_pool(name="expp", bufs=2) as expp, \
             tc.tile_pool(name="qpad", bufs=1) as qpp, \
             tc.tile_pool(name="rcbc", bufs=1) as rcp, \
             tc.tile_pool(name="aps", bufs=1, space="PSUM") as aps:

            zsc = qpp.tile([P, QS], F32)
            nc.vector.memset(zsc[:], 0.0)
            qpadA = [qpp.tile([P, QS], F32R, name=f"qpadA{i}") for i in range(1)]
            qpadB = [qpp.tile([P, QS], F32R, name=f"qpadB{i}") for i in range(1)]
            for i in range(1):
                nc.vector.tensor_copy(qpadA[i][:], zsc[:])
                nc.vector.tensor_copy(qpadB[i][:], zsc[:])

            QT_g = kvp.tile([P, 2, NQ], F32R)
            KT_g = kvp.tile([P, 2, S], F32R)
            # per (toktile, pair, head j): [V_head(64) | 1 | 0(63)]
            V_gp = kvp.tile([P, ST, 2, 2, P], F32R)
            for t in range(ST):
                nc.vector.tensor_copy(
                    V_gp[:, t], zsc[:].rearrange("p (a b m) -> p a b m", a=2, b=2))
            one_r = qpp.tile([P, 1], F32R)
            nc.vector.tensor_copy(one_r[:], ones_f[:, 0:1])
            for t in range(ST):
                for pi in range(2):
                    for j in range(2):
                        nc.vector.tensor_copy(V_gp[:, t, pi, j, 64:65], one_r[:])
            it_count = 0

            for g in range(NG):
                for pl in range(2):   # head pairs 2g, 2g+1
                    pr = 2 * g + pl
                    wq_t = wsp.tile([P, KD, P], F32R, tag="wq_t")
                    nc.sync.dma_start(wq_t[:], Wq[pr])
                    for q in range(NQS):
                        ps = aps.tile([P, QS], F32, tag="pp", bufs=2)
                        for kd in range(KD):
                            nc.tensor.matmul(
                                ps[:], wq_t[:, kd, :], xn_kvT[:, kd, q * QS:(q + 1) * QS],
                                start=(kd == 0), stop=(kd == KD - 1))
                        nc.vector.tensor_scalar_add(
                            QT_g[:, pl, q * QS:(q + 1) * QS], ps[:], bq_t[:, pr:pr + 1])
                    wk_t = wsp.tile([P, KD, P], F32R, tag="wk_t")
                    nc.sync.dma_start(wk_t[:], Wk[pr])
                    for q in range(NKS):
                        ps = aps.tile([P, QS], F32, tag="pp", bufs=2)
                        for kd in range(KD):
                            nc.tensor.matmul(
                                ps[:], wk_t[:, kd, :], xn_kvT[:, kd, q * QS:(q + 1) * QS],
                                start=(kd == 0), stop=(kd == KD - 1))
                        nc.vector.tensor_scalar_add(
                            KT_g[:, pl, q * QS:(q + 1) * QS], ps[:], bk_t[:, pr:pr + 1])
                wv_t = wsp.tile([P, KD, 256], F32R, tag="wv_t", bufs=1)
                nc.sync.dma_start(wv_t[:], Wv[g])
                for t in range(ST):
                    ps = aps.tile([P, QS], F32, tag="pp", bufs=2)
                    for kd in range(KD):
                        nc.tensor.matmul(
                            ps[:, 0:256], xn_kvT[:, kd, t * P:(t + 1) * P], wv_t[:, kd, :],
                            start=(kd == 0), stop=(kd == KD - 1))
                    for pi in range(2):
                        nc.vector.tensor_tensor(
                            V_gp[:, t, pi, :, 0:64],
                            ps[:, pi * 128:(pi + 1) * 128].rearrange("p (j m) -> p j m", j=2),
                            bv_rep[:, g * 256 + pi * 128:g * 256 + (pi + 1) * 128].rearrange(
                                "p (j m) -> p j m", j=2), ALU.add)

                if DEBUG and g == 0:
                    nc.sync.dma_start(dbg["kt0"], KT_g[:])
                    nc.sync.dma_start(dbg["v0"], V_gp[:])
                    nc.sync.dma_start(dbg["qt0"], QT_g[:])

                for q in range(NQS):
                    for pl in range(2):
                        pr = 2 * g + pl
                        i = it_count % 1
                        it_count += 1
                        qsl = slice(q * QS, (q + 1) * QS)
                        nc.vector.tensor_copy(qpadA[i][0:64, :], QT_g[0:64, pl, qsl])
                        nc.vector.tensor_copy(qpadB[i][64:128, :], QT_g[64:128, pl, qsl])
                        av1 = aps.tile([P, QS], F32, tag="av1")
                        av2 = aps.tile([P, QS], F32, tag="av2")
                        for kt in range(ST):
                            ktsl = slice(kt * P, (kt + 1) * P)
                            sAB = aps.tile([P, 2, QS], F32, tag="sAB", bufs=2)
                            nc.tensor.matmul(sAB[:, 0, :], KT_g[:, pl, ktsl], qpadA[i][:],
                                             start=True, stop=True)
                            nc.tensor.matmul(sAB[:, 1, :], KT_g[:, pl, ktsl], qpadB[i][:],
                                             start=True, stop=True)
                            eAB = expp.tile([P, 2, QS], F32R, tag="eAB")
                            nc.scalar.activation(eAB[:], sAB[:], AF.Exp, scale=0.125)
                            eA = eAB[:, 0, :]
                            eB = eAB[:, 1, :]
                            if DEBUG and g == 0 and q == 0 and pl == 0 and kt == 0:
                                nc.sync.dma_start(dbg["e0"], eA)
                                s0c = rcp.tile([P, QS], F32, tag="s0c")
                                nc.vector.tensor_copy(s0c[:], sAB[:, 0, :])
                                nc.sync.dma_start(dbg["s0"], s0c[:])
                            st, sp = (kt == 0), (kt == ST - 1)
                            nc.tensor.matmul(av1[:], V_gp[:, kt, pl, 0, :], eA,
                                             start=st, stop=sp, skip_group_check=True)
                            nc.tensor.matmul(av2[:], V_gp[:, kt, pl, 1, :], eB,
                                             start=st, stop=sp, skip_group_check=True)
                        # free the av psums fast: copy to SBUF, divide from there
                        avc = rcp.tile([65, 2, QS], F32, tag="avc")
                        nc.vector.tensor_copy(avc[0:65, 0, :], av1[0:65, :])
                        nc.vector.tensor_copy(avc[0:65, 1, :], av2[0:65, :])
                        nc.vector.reciprocal(avc[64:65, 0, :], avc[64:65, 0, :])
                        nc.vector.reciprocal(avc[64:65, 1, :], avc[64:65, 1, :])
                        rcd = dramp.tile([2, QS], F32, tag="rcd", bufs=2)
                        nc.sync.dma_start(rcd[0:1, :], avc[64:65, 0, :])
                        nc.sync.dma_start(rcd[1:2, :], avc[64:65, 1, :])
                        bcA = rcp.tile([64, QS], F32, tag="bcA")
                        bcB = rcp.tile([64, QS], F32, tag="bcB")

                        def _b64(row_ap):
                            return bass.AP(tensor=row_ap.tensor, offset=row_ap.offset,
                                           ap=[[0, 64]] + list(row_ap.ap)[1:])

                        nc.sync.dma_start(bcA[:], _b64(rcd[0:1, :]))
                        nc.sync.dma_start(bcB[:], _b64(rcd[1:2, :]))
                        if DEBUG and g == 0 and q == 0 and pl == 0:
                            nc.sync.dma_start(dbg["av0"], avc[:])
                        nc.vector.tensor_tensor(RT_h[q][0:64, pr, :], avc[0:64, 0, :], bcA[:], ALU.mult)
                        stB = rcp.tile([64, QS], F32R, tag="stB")
                        nc.vector.tensor_tensor(stB[:], avc[0:64, 1, :], bcB[:], ALU.mult)
                        nc.sync.dma_start(RT_h[q][64:128, pr, :], stB[:])

        xn_es.close()

        if DEBUG:
            for h in range(NQS):
                nc.sync.dma_start(
                    dbg["rt"].rearrange("p k (h w) -> p k h w", h=NQS)[:, :, h, :], RT_h[h][:])

        # ---- Phase 4a: O-projection + residual -> x2 (DRAM) ----
        with tc.tile_pool(name="p4tmp", bufs=2) as p4t, \
             tc.tile_pool(name="p4ps", bufs=2, space="PSUM") as ps4, \
             tc.tile_pool(name="p4tps", bufs=6, space="PSUM") as ps4t:
            for q in range(NQS):
                attnT = p4t.tile([P, KD, QS], F32R, tag="attnT")
                for mt in range(KD):
                    wo_t = p4t.tile([P, KD, P], F32R, tag="wo_t")
                    nc.sync.dma_start(wo_t[:], Wo[mt])
                    ps = ps4.tile([P, QS], F32, tag="pp")
                    for kd in range(KD):
                        nc.tensor.matmul(
                            ps[:], wo_t[:, kd, :], RT_h[q][:, kd, :],
                            start=(kd == 0), stop=(kd == KD - 1))
                    nc.vector.tensor_scalar_add(
                        attnT[:, mt, :], ps[:], bo_t[:, mt:mt + 1])
                for j in range(QS // P):
                    tt = q * (QS // P) + j
                    xr_t = p4t.tile([P, D], F32, tag="xr_t")
                    nc.sync.dma_start(xr_t[:], xkv[tt * P:(tt + 1) * P, :])
                    x2_t = p4t.tile([P, D], F32, tag="x2_t")
                    for m2 in range(KD // 2):
                        pst = ps4t.tile([P, 2, P], F32, tag="tp")
                        for h in range(2):
                            nc.tensor.transpose(
                                pst[:, h, :].bitcast(F32R),
                                attnT[:, 2 * m2 + h, j * P:(j + 1) * P], ident[:])
                        nc.vector.tensor_tensor(
                            x2_t[:, 2 * m2 * P:(2 * m2 + 2) * P],
                            pst[:].rearrange("p a m -> p (a m)"),
                            xr_t[:, 2 * m2 * P:(2 * m2 + 2) * P], ALU.add)
                    nc.sync.dma_start(x2d[:, tt, :], x2_t[:])
                    if DEBUG:
                        nc.sync.dma_start(dbg["x2"][:, tt, :], x2_t[:])
        rt_es.close()

        # ---- Phase 4b: LN2 -> xn2T ----
        xn2_es = ExitStack()
        xn2p = xn2_es.enter_context(tc.tile_pool(name="xn2", bufs=1))
        xn2T_h = [xn2p.tile([P, KD, QS], F32R, name=f"xn2T{h}") for h in range(NQS)]
        with tc.tile_pool(name="p4btmp", bufs=4) as p4bt, \
             tc.tile_pool(name="p4bs", bufs=4) as p4bs, \
             tc.tile_pool(name="ln2", bufs=1) as ln2p, \
             tc.tile_pool(name="p4bps", bufs=6, space="PSUM") as ps4b:
            g2_rep = ln2p.tile([P, D], F32)
            nc.gpsimd.dma_start(g2_rep[:], bcast_ap(g2))
            be2_rep = ln2p.tile([P, D], F32)
            nc.gpsimd.dma_start(be2_rep[:], bcast_ap(be2))
            eps2_t = ln2p.tile([P, 1], F32)
            nc.vector.memset(eps2_t[:], EPS)

            for tt in range(QTT):
                x2_t = p4bt.tile([P, D], F32, tag="x2_t")
                nc.sync.dma_start(x2_t[:], x2d[:, tt, :])
                stats = p4bs.tile([P, 2, 6], F32, tag="stats2")
                xv = x2_t[:].rearrange("p (s f) -> p s f", s=2)
                for s in range(2):
                    nc.vector.bn_stats(stats[:, s, :], xv[:, s, :])
                mv = p4bs.tile([P, 2], F32, tag="mv2")
                nc.vector.bn_aggr(mv[:], stats[:])
                std = p4bs.tile([P, 1], F32, tag="std2")
                nc.scalar.activation(std[:], mv[:, 1:2], AF.Sqrt, bias=eps2_t[:])
                nc.vector.reciprocal(std[:], std[:])
                xn2_t = p4bt.tile([P, D], F32R, tag="xn2_t")
                nc.vector.tensor_scalar(
                    xn2_t[:], x2_t[:], scalar1=mv[:, 0:1], scalar2=std[:],
                    op0=ALU.subtract, op1=ALU.mult)
                if ln_affine:
                    nc.vector.tensor_tensor(xn2_t[:], xn2_t[:], g2_rep[:], ALU.mult)
                    nc.vector.tensor_tensor(xn2_t[:], xn2_t[:], be2_rep[:], ALU.add)
                hs_i, loc = tt // (QS // P), (tt % (QS // P)) * P
                for j2 in range(KD // 2):
                    pst = ps4b.tile([P, 2, P], F32, tag="tp")
                    for h in range(2):
                        nc.tensor.transpose(
                            pst[:, h, :].bitcast(F32R),
                            xn2_t[:, (2 * j2 + h) * P:(2 * j2 + h + 1) * P], ident[:])
                    nc.vector.tensor_copy(
                        xn2T_h[hs_i][:, 2 * j2:2 * j2 + 2, loc:loc + P], pst[:])

        # ---- Phase 5: MLP (h1 in bf16, single full-width token pass) ----
        with tc.tile_pool(name="p5tmp", bufs=3) as p5t, \
             tc.tile_pool(name="h1", bufs=1) as h1p, \
             tc.tile_pool(name="w2st", bufs=2) as w2p, \
             tc.tile_pool(name="p5ps", bufs=2, space="PSUM") as ps5, \
             tc.tile_pool(name="p5tps", bufs=4, space="PSUM") as ps5t:
            mdt = BF16 if mlp_bf16 else F32R
            n_hslice = 1 if mlp_bf16 else NQS
            HW_ = NQ // n_hslice
            out_acc = h1p.tile([P, QTT, D], F32)
            for hs in range(n_hslice):
                h1T = h1p.tile([P, FT, HW_], mdt, tag="h1T")
                for ft in range(FT):
                    w1_t = p5t.tile([P, KD, P], F32R, tag="w1_t")
                    nc.sync.dma_start(w1_t[:], W1[ft])
                    for sl in range(HW_ // QS):
                        gsl = (hs * HW_ + sl * QS) // QS
                        ps = ps5.tile([P, QS], F32, tag="pp")
                        for kd in range(KD):
                            nc.tensor.matmul(
                                ps[:], w1_t[:, kd, :], xn2T_h[gsl][:, kd, :],
                                start=(kd == 0), stop=(kd == KD - 1))
                        nc.scalar.activation(h1T[:, ft, sl * QS:(sl + 1) * QS], ps[:],
                                             AF.Gelu, bias=b1_t[:, ft:ft + 1])
                for mt in range(KD):
                    w2_t = w2p.tile([P, FT, P], mdt, tag="w2_t")
                    nc.sync.dma_start(w2_t[:], W2[mt])
                    for sl in range(HW_ // QS):
                        ssl_loc = slice(sl * QS, (sl + 1) * QS)
                        ps = ps5.tile([P, QS], F32, tag="pp")
                        for ft in range(FT):
                            nc.tensor.matmul(
                                ps[:], w2_t[:, ft, :], h1T[:, ft, ssl_loc],
                                start=(ft == 0), stop=(ft == FT - 1))
                        outT = p5t.tile([P, QS], F32R, tag="outT", bufs=2)
                        nc.vector.tensor_scalar_add(outT[:], ps[:], b2_t[:, mt:mt + 1])
                        for j in range(QS // P):
                            tt = hs * (HW_ // P) + sl * (QS // P) + j
                            pst = ps5t.tile([P, P], F32, tag="tp")
                            nc.tensor.transpose(pst[:].bitcast(F32R),
                                                outT[:, j * P:(j + 1) * P], ident[:])
                            nc.vector.tensor_copy(out_acc[:, tt, mt * P:(mt + 1) * P], pst[:])
            for tt in range(QTT):
                x2_t = p5t.tile([P, D], F32, tag="x2r_t")
                nc.sync.dma_start(x2_t[:], x2d[:, tt, :])
                ob = p5t.tile([P, D], F32, tag="ob")
                nc.vector.tensor_tensor(ob[:], out_acc[:, tt, :], x2_t[:], ALU.add)
                nc.sync.dma_start(out[tt * P:(tt + 1) * P, :], ob[:])

        xn2_es.close()
        es.close()

    nc.compile()
    return nc


def kernel(**inputs):
    inputs = {k: np.ascontiguousarray(np.asarray(v), dtype=np.float32)
              for k, v in inputs.items()}
    ln_affine = not (
        np.all(inputs["ln1_g"] == 1.0) and np.all(inputs["ln1_b"] == 0.0)
        and np.all(inputs["ln2_g"] == 1.0) and np.all(inputs["ln2_b"] == 0.0))
    key = ("nc", ln_affine, MLP_BF16)
    if key not in _CACHE:
        _CACHE[key] = _build(ln_affine=ln_affine, mlp_bf16=MLP_BF16)
    nc = _CACHE[key]

    x = inputs["x"]
    def tile_w(W, n_out, m):
        # [Din, Dout] -> [Dout/m, 128, Din/128, m]
        Din, Dout = W.shape
        return np.ascontiguousarray(
            W.reshape(Din // P, P, n_out, m).transpose(2, 1, 0, 3))

    shared = {
        "Wq": tile_w(inputs["Wq"], KD, P), "Wk": tile_w(inputs["Wk"], KD, P),
        "Wv": tile_w(inputs["Wv"], NG, 256), "Wo": tile_w(inputs["Wo"], KD, P),
        "W1": tile_w(inputs["W1"], FT, P),
        "W2": (tile_w(inputs["W2"], KD, P).astype(__import__("ml_dtypes").bfloat16)
               if MLP_BF16 else tile_w(inputs["W2"], KD, P)),
        "bq": inputs["bq"], "bk": inputs["bk"], "bv": inputs["bv"], "bo": inputs["bo"],
        "b1": inputs["b1"], "b2": inputs["b2"],
        "g1": inputs["ln1_g"], "be1": inputs["ln1_b"],
        "g2": inputs["ln2_g"], "be2": inputs["ln2_b"],
    }
    in_maps = []
    for c in range(8):
        b, half = c // 2, c % 2
        m = dict(shared)
        # query half first; attention is permutation-invariant over kv order
        m["xkv"] = np.ascontiguousarray(
            np.concatenate([x[b, half * NQ:(half + 1) * NQ, :],
                            x[b, (1 - half) * NQ:(2 - half) * NQ, :]], axis=0))
        in_maps.append(m)

    trace = bool(int(os.environ.get("KERNEL_TRACE", "0")))
    kw = {}
    if trace:
        kw = dict(trace=True, tmpdir=os.environ.get("KERNEL_TRACE_DIR") or None)
    res = bass_utils.run_bass_kernel_spmd(nc, in_maps, core_ids=list(range(8)), **kw)
    _CACHE["last_results"] = res
    _CACHE["nc"] = nc
    _CACHE["last_in_maps"] = in_maps

    outa = np.empty((B, S, D), dtype=np.float32)
    for c in range(8):
        b, half = c // 2, c % 2
        outa[b, half * NQ:(half + 1) * NQ, :] = res.results[c]["out"]
    return outa



# revision 35
# speedup vs baseline: 1.5178x; 1.5178x over previous
"""Trainium2 Bass kernel for a dense transformer encoder block (B=4, S=2048,
D=1024, H=16, MLP=4096), fp8-e4m3 DoubleRow edition.

Sharding: 8 cores = 4 batch x 2 query-halves, no collectives (as the f32r
baseline: kv host-reordered query-half-first; K/V computed for all 2048
tokens per core).

All matmuls run as fp8e4 DoubleRow (256-deep contraction, 0.5 cycles/row =
4x f32r in the cost model; ~1.4x on silicon):
  - weights host-quantized at x32 (W2 x64) so values sit in e4m3 normal range
  - QKV/O projections and attention plain fp8 (study err contribution ~2e-3)
  - scores: zero-slot DoubleRow (K tile in slot 0, zeros in slot 1) since the
    per-head contraction is only DH=64; 2x the f32r rate
  - AV: V|ones 65-column lhsT over kt-pairs, full 4x rate
  - softmax: e' = exp(s/8 - 3) stored fp8 by ACT; the constant shift cancels
    in the normalization. C=3 keeps e' in fp8 normal range (ACT fp8 writes
    flush subnormals) with overflow margin to s=8.4 (max observed ~6.3)
  - MLP: both layers 3-term split fp8 (W_hi@x_hi + W_lo@x_hi + W_hi@x_lo);
    host-side W splits keep subnormals (PE reads them fine), device-side
    activation splits done by DVE/Pool (subnormal-exact, unlike ACT)
End-to-end rel err ~5e-3 vs the f32 reference (budget 2e-2).

Engine split: PE matmuls/transposes; ACT exp + gelu + LN sqrt; DVE all
PSUM-consuming quantize/normalize ops (GPSIMD cannot touch PSUM); Pool
(gpsimd) SBUF-only work: LN normalize, h1 hi/lo split, xn_kvT copyback.
"""

import os
import sys

sys.path.insert(0, "/opt/trn_rl_repo")

from contextlib import ExitStack

import numpy as np
import ml_dtypes

import concourse.bass as bass
import concourse.tile as tile
from concourse import bacc, bass_utils, mybir
from concourse.masks import make_identity

F32 = mybir.dt.float32
BF16 = mybir.dt.bfloat16
FP8 = mybir.dt.float8e4
AF = mybir.ActivationFunctionType
ALU = mybir.AluOpType
DR = mybir.MatmulPerfMode.DoubleRow
E4 = ml_dtypes.float8_e4m3
BFm = ml_dtypes.bfloat16

B, S, D = 4, 2048, 1024
H, DH, MLP = 16, 64, 4096
P = 128
KD = D // P            # 8 feature tiles over D
FT = MLP // P          # 32 feature tiles over MLP
NQ = S // 2            # 1024 queries per core
ST = S // P            # 16 kv token tiles
NG = 4                 # head groups (2 pairs each)
NC = 4                 # 256-wide contraction chunks over D
C_EXP = 3.0            # softmax shift (cancels in normalization)
WS = 32.0              # weight pre-scale for fp8 range
EPS = 1e-6

_CACHE = {}


def _build(ln_affine, has_bv, has_qkb):
    nc = bacc.Bacc(None, target_bir_lowering=False, debug=False, num_devices=8)

    xkv = nc.dram_tensor("xkv", [S, D], F32, kind="ExternalInput").ap()
    # DoubleRow weight tiles: [out_tile, 128p, chunk, 2, m]
    Wq = nc.dram_tensor("Wq", [KD, P, NC, 2, P], FP8, kind="ExternalInput").ap()
    Wk = nc.dram_tensor("Wk", [KD, P, NC, 2, P], FP8, kind="ExternalInput").ap()
    Wv = nc.dram_tensor("Wv", [NG, P, NC, 2, 256], FP8, kind="ExternalInput").ap()
    Wo = nc.dram_tensor("Wo", [KD, P, NC, 2, P], FP8, kind="ExternalInput").ap()
    # fc weights with hi/lo split packed on a trailing pair axis:
    # [out_tile, 128p, chunk, i, {hi,lo}, m]
    W1hl = nc.dram_tensor("W1hl", [FT, P, NC, 2, 2, P], FP8, kind="ExternalInput").ap()
    W2hl = nc.dram_tensor("W2hl", [KD, P, FT // 2, 2, 2, P], FP8, kind="ExternalInput").ap()
    bq = nc.dram_tensor("bq32", [D], F32, kind="ExternalInput").ap()   # x32
    bk = nc.dram_tensor("bk32", [D], F32, kind="ExternalInput").ap()   # x32
    bv = nc.dram_tensor("bv32", [D], F32, kind="ExternalInput").ap()   # x32
    bo = nc.dram_tensor("bo", [D], F32, kind="ExternalInput").ap()
    b1 = nc.dram_tensor("b1", [MLP], F32, kind="ExternalInput").ap()
    b2 = nc.dram_tensor("b2", [D], F32, kind="ExternalInput").ap()
    g1 = nc.dram_tensor("g1", [D], F32, kind="ExternalInput").ap()
    be1 = nc.dram_tensor("be1", [D], F32, kind="ExternalInput").ap()
    g2 = nc.dram_tensor("g2", [D], F32, kind="ExternalInput").ap()
    be2 = nc.dram_tensor("be2", [D], F32, kind="ExternalInput").ap()
    out = nc.dram_tensor("out", [NQ, D], F32, kind="ExternalOutput").ap()
    DEBUG = bool(int(os.environ.get("KERNEL_DEBUG", "0")))
    dbg = {}
    if DEBUG:
        for nm, shape, dt in [
                ("d_xn", [P, KD, S], FP8), ("d_kt", [P, KD, ST + 1, P], FP8),
                ("d_qa", [P, KD, NQ + 256], FP8), ("d_qb", [P, KD, NQ + 256], FP8),
                ("d_v", [P, ST, NG, 2, 2, 80], FP8),
                ("d_e", [P, 2, 2, 256], FP8), ("d_rt", [P, KD, NQ], FP8),
                ("d_x2", [P, 2, D], F32), ("d_h1h", [P, 4, 256], FP8),
                ("d_h1l", [P, 4, 256], FP8), ("d_std", [P, ST], F32)]:
            dbg[nm] = nc.dram_tensor(nm, shape, dt, kind="ExternalOutput").ap()

    def bcast_ap(vec, n=P):
        return bass.AP(tensor=vec.tensor, offset=vec.offset,
                       ap=[[0, n]] + list(vec.ap))

    with tile.TileContext(nc) as tc:
        es = ExitStack()
        params = es.enter_context(tc.tile_pool(name="params", bufs=1))
        dramp = es.enter_context(tc.tile_pool(name="dram", bufs=1, space="DRAM"))

        ident_f = params.tile([P, P], F32)
        make_identity(nc, ident_f)
        ident = params.tile([P, P], BF16)
        nc.vector.tensor_copy(ident[:], ident_f[:])

        def pvec(v, n, nm):  # [n*128] -> [128, n]
            t = params.tile([P, n], F32, name=nm)
            nc.sync.dma_start(t[:], v.rearrange("(o p) -> p o", p=P))
            return t

        bq_t = pvec(bq, KD, "bq_t")
        bk_t = pvec(bk, KD, "bk_t")
        bo_t = pvec(bo, KD, "bo_t")
        b2_t = pvec(b2, KD, "b2_t")
        b1_t = pvec(b1, FT, "b1_t")
        eps_t = params.tile([P, 1], F32)
        nc.vector.memset(eps_t[:], EPS)
        one_i = params.tile([P, 1], mybir.dt.int32)
        nc.vector.memset(one_i[:], 1)
        magic_i = params.tile([P, 1], mybir.dt.int32)
        nc.vector.memset(magic_i[:], 0x5f3759df)
        expb_t = params.tile([P, 1], F32)
        nc.vector.memset(expb_t[:], -C_EXP)
        if has_bv:
            bv_rep = params.tile([P, D], F32)
            nc.gpsimd.dma_start(bv_rep[:], bcast_ap(bv))
        if ln_affine:
            g1_rep = params.tile([P, D], F32)
            nc.gpsimd.dma_start(g1_rep[:], bcast_ap(g1))
            be1_rep = params.tile([P, D], F32)
            nc.gpsimd.dma_start(be1_rep[:], bcast_ap(be1))
            g2_rep = params.tile([P, D], F32)
            nc.gpsimd.dma_start(g2_rep[:], bcast_ap(g2))
            be2_rep = params.tile([P, D], F32)
            nc.gpsimd.dma_start(be2_rep[:], bcast_ap(be2))

        # pools ordered by lifetime (LIFO close discipline)
        rt_es = ExitStack()
        rtp = rt_es.enter_context(tc.tile_pool(name="rt", bufs=1))
        RT = rtp.tile([P, KD, NQ], FP8)
        kv_es = ExitStack()
        kvp = kv_es.enter_context(tc.tile_pool(name="kv", bufs=1))
        xn_es = ExitStack()
        xnp = xn_es.enter_context(tc.tile_pool(name="xn", bufs=1))
        xn_kvT = xnp.tile([P, KD, S], FP8)

        dbg_state = {"t": 0}

        def layernorm_tile(x_ap, pool, tag, gr, br):
            """x_ap [P, D] f32 -> bf16 normalized (Pool does the scale)."""
            stats = pool.tile([P, 2, 6], F32, tag=tag + "st")
            xv = x_ap.rearrange("p (s f) -> p s f", s=2)
            for s2 in range(2):
                nc.vector.bn_stats(stats[:, s2, :], xv[:, s2, :])
            mv = pool.tile([P, 2], F32, tag=tag + "mv")
            nc.vector.bn_aggr(mv[:], stats[:])
            # rstd = (var+eps)^-0.5 via the bit-trick rsqrt + one Newton
            # step, all on DVE: keeps LayerNorm off the ACT engine entirely
            # (no act-table switches against exp/gelu); rel err ~2e-3, far
            # below the fp8 quantization noise downstream
            I32 = mybir.dt.int32
            vpe = pool.tile([P, 1], F32, tag=tag + "ve")
            nc.vector.tensor_scalar_add(vpe[:], mv[:, 1:2], EPS)
            vh = pool.tile([P, 1], F32, tag=tag + "vh")
            nc.vector.tensor_scalar_mul(vh[:], vpe[:], 0.5)
            ti = pool.tile([P, 1], I32, tag=tag + "ti")
            nc.vector.tensor_tensor(ti[:], vpe[:].bitcast(I32), one_i[:],
                                    ALU.logical_shift_right)
            y0i = pool.tile([P, 1], I32, tag=tag + "y0")
            nc.vector.tensor_tensor(y0i[:], magic_i[:], ti[:], ALU.subtract)
            y0 = y0i[:].bitcast(F32)
            w = pool.tile([P, 1], F32, tag=tag + "w")
            nc.vector.tensor_tensor(w[:], y0, y0, ALU.mult)
            nc.vector.tensor_tensor(w[:], vh[:], w[:], ALU.mult)
            nc.vector.tensor_scalar(w[:], w[:], scalar1=-1.0, scalar2=1.5,
                                    op0=ALU.mult, op1=ALU.add)
            std = pool.tile([P, 1], F32, tag=tag + "sd")
            nc.vector.tensor_tensor(std[:], y0, w[:], ALU.mult)
            if DEBUG and tag == "l1":
                nc.sync.dma_start(dbg["d_std"][:, dbg_state["t"]:dbg_state["t"] + 1], std[:])
                dbg_state["t"] += 1
            xn_b = pool.tile([P, D], BF16, tag=tag + "nb")
            nc.gpsimd.tensor_scalar(
                xn_b[:], x_ap, scalar1=mv[:, 0:1], scalar2=std[:],
                op0=ALU.subtract, op1=ALU.mult)
            if ln_affine:
                nc.gpsimd.tensor_tensor(xn_b[:], xn_b[:], gr[:], ALU.mult)
                nc.gpsimd.tensor_tensor(xn_b[:], xn_b[:], br[:], ALU.add)
            return xn_b

        # ---- Phase A: LN1 interleaved with QKV projections ----
        # K feature-major, +1 garbage kt slot so the dual-slot lhsT [kt, kt+1]
        # stays in-bounds at kt=15 (its product is zeroed by the rhs zero tail)
        KT = kvp.tile([P, KD, ST + 1, P], FP8)
        nc.vector.memset(KT[:, :, ST, :], 0.0)
        # V with ones column at 64 (pad to 80 for the 16B lhsT step rule)
        V_all = kvp.tile([P, ST, NG, 2, 2, 80], FP8)
        nc.vector.memset(V_all[:, :, :, :, :, 64:65], 1.0)
        # Q zero-padded per head half; tail [NQ:NQ+256] stays zero and serves
        # as the dual-slot zero region for the scores rhs
        QpA = kvp.tile([P, KD, NQ + 256], FP8)
        QpB = kvp.tile([P, KD, NQ + 256], FP8)
        nc.vector.memset(QpA[:], 0.0)
        nc.gpsimd.memset(QpB[:], 0.0)

        with tc.tile_pool(name="p1t", bufs=3) as p1t, \
             tc.tile_pool(name="p1s", bufs=3) as p1s, \
             tc.tile_pool(name="pav", bufs=1) as pavw, \
             tc.tile_pool(name="p1ps", bufs=4, space="PSUM") as ps1, \
             tc.tile_pool(name="paps", bufs=1, space="PSUM") as paps:
            wq_t = pavw.tile([P, KD, NC, 2, P], FP8)
            nc.sync.dma_start(wq_t[:], Wq.rearrange("o p c i m -> p o c i m"))
            wk_t = pavw.tile([P, KD, NC, 2, P], FP8)
            nc.sync.dma_start(wk_t[:], Wk.rearrange("o p c i m -> p o c i m"))
            wv_t = pavw.tile([P, NG, NC, 2, 256], FP8)
            nc.sync.dma_start(wv_t[:], Wv.rearrange("o p c i m -> p o c i m"))

            for t in range(ST):
                x_t = p1t.tile([P, D], F32, tag="x_t")
                nc.sync.dma_start(x_t[:], xkv[t * P:(t + 1) * P, :])
                xn_b = layernorm_tile(x_t[:], p1s, "l1",
                                      g1_rep if ln_affine else None,
                                      be1_rep if ln_affine else None)
                for j2 in range(KD // 2):
                    pst = ps1.tile([P, 2, P], BF16, tag="tp")
                    for h in range(2):
                        nc.tensor.transpose(
                            pst[:, h, :],
                            xn_b[:, (2 * j2 + h) * P:(2 * j2 + h + 1) * P],
                            ident[:])
                    nc.scalar.copy(
                        xn_kvT[:, 2 * j2:2 * j2 + 2, t * P:(t + 1) * P], pst[:])
                # V projection for this token tile (all groups)
                ps = paps.tile([P, 512], F32, tag="pp", bufs=2)
                for g in range(NG):
                    for c in range(NC):
                        nc.tensor.matmul(
                            ps[:, 0:256],
                            xn_kvT[:, 2 * c:2 * c + 2, t * P:(t + 1) * P],
                            wv_t[:, g, c],
                            start=(c == 0), stop=(c == NC - 1), perf_mode=DR)
                    psv = ps[:, 0:256].rearrange("p (a j m) -> p a j m", a=2, j=2)
                    if has_bv:
                        nc.vector.tensor_tensor(
                            V_all[:, t, g, :, :, 0:64], psv,
                            bv_rep[:, g * 256:(g + 1) * 256].rearrange(
                                "p (a j m) -> p a j m", a=2, j=2), ALU.add)
                    else:
                        nc.scalar.copy(V_all[:, t, g, :, :, 0:64], psv)
                    if g < NG - 1:
                        ps = paps.tile([P, 512], F32, tag="pp", bufs=2)
                # K (and for the first half Q) projections per 4-tile group
                if t % 4 == 3:
                    ks = t // 4
                    ksl = slice(ks * 512, (ks + 1) * 512)
                    for pr in range(KD):
                        ps = paps.tile([P, 512], F32, tag="pp", bufs=2)
                        for c in range(NC):
                            nc.tensor.matmul(
                                ps[:], wk_t[:, pr, c],
                                xn_kvT[:, 2 * c:2 * c + 2, ksl],
                                start=(c == 0), stop=(c == NC - 1), perf_mode=DR)
                        nc.vector.tensor_scalar_add(
                            KT[:, pr, 4 * ks:4 * ks + 4, :],
                            ps[:].rearrange("p (k m) -> p k m", k=4),
                            bk_t[:, pr:pr + 1])
                    if ks < 2:
                        for pr in range(KD):
                            ps = paps.tile([P, 512], F32, tag="pp", bufs=2)
                            for c in range(NC):
                                nc.tensor.matmul(
                                    ps[:], wq_t[:, pr, c],
                                    xn_kvT[:, 2 * c:2 * c + 2, ksl],
                                    start=(c == 0), stop=(c == NC - 1),
                                    perf_mode=DR)
                            if has_qkb:
                                nc.vector.tensor_scalar_add(
                                    QpA[0:64, pr, ksl], ps[0:64, :],
                                    bq_t[0:64, pr:pr + 1])
                                nc.vector.tensor_scalar_add(
                                    QpB[64:128, pr, ksl], ps[64:128, :],
                                    bq_t[64:128, pr:pr + 1])
                            else:
                                nc.scalar.copy(QpA[0:64, pr, ksl], ps[0:64, :])
                                nc.scalar.copy(QpB[64:128, pr, ksl],
                                               ps[64:128, :])
        if DEBUG:
            nc.sync.dma_start(dbg["d_xn"], xn_kvT[:])
            nc.sync.dma_start(dbg["d_kt"], KT[:])
            nc.sync.dma_start(dbg["d_qa"], QpA[:])
            nc.sync.dma_start(dbg["d_qb"], QpB[:])
            nc.sync.dma_start(dbg["d_v"], V_all[:])
        xn_es.close()


        # ---- Phase B: pipelined attention(q) || O-proj/LN2/MLP(q-1) over
        # four 256-query quarters (interleaved emission keeps the ACT-bound
        # softmax overlapped with the PE-bound MLP) ----
        def bcast8(col):
            return bass.AP(tensor=col.tensor, offset=col.offset,
                           ap=[col.ap[0], [0, KD], [1, 1]])

        def zq_rhs(QT, pr, q0):
            # scores rhs: dual slot i=0 -> 256 real queries at q0, slot i=1 ->
            # the zero tail at NQ (multiplied into the real K of slot kt+1)
            base = QT[:, pr, :]
            return bass.AP(tensor=base.tensor, offset=base.offset + q0,
                           ap=[base.ap[0], [NQ - q0, 2], [1, 256]])

        bf_es = ExitStack()
        expp = bf_es.enter_context(tc.tile_pool(name="expp", bufs=2))
        rcp = bf_es.enter_context(tc.tile_pool(name="rcb", bufs=2))
        mwp = bf_es.enter_context(tc.tile_pool(name="mw", bufs=2))
        mtp = bf_es.enter_context(tc.tile_pool(name="mt", bufs=2))
        mqp = bf_es.enter_context(tc.tile_pool(name="mq", bufs=1))
        aps = bf_es.enter_context(tc.tile_pool(name="bps", bufs=1, space="PSUM"))
        tps = bf_es.enter_context(tc.tile_pool(name="btps", bufs=1, space="PSUM"))

        def gen_attn(qq):
            q0 = qq * 256
            qsl = slice(q0, q0 + 256)
            for pr in range(KD):
                g, pl = pr // 2, pr % 2
                av = aps.tile([P, 2, 256], F32, tag="av", bufs=1)
                e8s = [None] * 8
                # scores/exp for chunk k+1 are emitted before AV of chunk k so
                # the PE never head-of-line-blocks the ACT exp stream
                for ktp in range(9):
                    if ktp < 8:
                        sAB = aps.tile([P, 2, 2, 256], F32, tag="sAB", bufs=2)
                        for kt2 in range(2):
                            kt = 2 * ktp + kt2
                            for hd, QT in ((0, QpA), (1, QpB)):
                                nc.tensor.matmul(
                                    sAB[:, kt2, hd, :], KT[:, pr, kt:kt + 2, :],
                                    zq_rhs(QT, pr, q0),
                                    start=True, stop=True, perf_mode=DR,
                                    skip_group_check=True)
                        e8 = expp.tile([P, 2, 2, 256], FP8, tag="e8")
                        nc.scalar.activation(e8[:], sAB[:], AF.Exp,
                                             bias=expb_t[:],
                                             scale=1.0 / (WS * WS * 8.0))
                        e8s[ktp] = e8
                        if DEBUG and qq == 0 and pr == 0 and ktp == 0:
                            nc.sync.dma_start(dbg["d_e"], e8[:])
                    if ktp > 0:
                        kp = ktp - 1
                        for hd in range(2):
                            nc.tensor.matmul(
                                av[0:65, hd, :],
                                V_all[:, 2 * kp:2 * kp + 2, g, pl, hd, 0:65],
                                e8s[kp][:, :, hd, :],
                                start=(kp == 0), stop=(kp == 7),
                                perf_mode=DR, skip_group_check=True)
                avc = rcp.tile([65, 2, 256], F32, tag="avc")
                nc.vector.tensor_copy(avc[:], av[0:65, :, :])
                rd = rcp.tile([1, 2, 256], F32, tag="rd")
                nc.vector.reciprocal(rd[:], avc[64:65, :, :])
                rcd = dramp.tile([1, 2, 256], F32, tag="rcd", bufs=2)
                nc.sync.dma_start(rcd[:], rd[:])
                bc = rcp.tile([64, 2, 256], F32, tag="bc")
                nc.sync.dma_start(bc[:], bcast_ap(rcd[0, :, :], 64))
                nc.vector.tensor_tensor(
                    RT[0:64, pr, qsl], avc[0:64, 0, :], bc[:, 0, :], ALU.mult)
                stB = rcp.tile([64, 256], FP8, tag="stB")
                nc.vector.tensor_tensor(
                    stB[:], avc[0:64, 1, :], bc[:, 1, :], ALU.mult)
                nc.gpsimd.dma_start(RT[64:128, pr, qsl], stB[:])
                if pr == KD - 1:
                    gens_state[("e8", qq)] = e8s[7]
                    if DEBUG and qq == 3:
                        nc.sync.dma_start(dbg["d_rt"], RT[:])
                yield

        def gen_mlp_a(qq):
            q0 = qq * 256
            qsl = slice(q0, q0 + 256)
            tpb = tps.tile([P, 4, P], BF16, tag="tp")
            tslot = [0]

            def tpair():
                s = tslot[0]
                tslot[0] = (s + 2) % 4
                return tpb[:, s:s + 2, :]

            # O-projection
            attnT = mqp.tile([P, KD, 256], BF16, tag="attnT")
            wo_tiles = {}

            def wo_fetch(mt):
                t = mwp.tile([P, NC, 2, P], FP8, tag="wo_t", bufs=3)
                nc.sync.dma_start(t[:], Wo[mt])
                wo_tiles[mt] = t

            wo_fetch(0)
            wo_fetch(1)
            for mt in range(KD):
                if mt + 2 < KD:
                    wo_fetch(mt + 2)
                wo_t = wo_tiles.pop(mt)
                ps = aps.tile([P, 256], F32, tag="mpp", bufs=2)
                for c in range(NC):
                    nc.tensor.matmul(
                        ps[:], wo_t[:, c], RT[:, 2 * c:2 * c + 2, qsl],
                        start=(c == 0), stop=(c == NC - 1), perf_mode=DR)
                nc.vector.tensor_scalar(
                    attnT[:, mt, :], ps[:],
                    scalar1=1.0 / (WS * WS), scalar2=bo_t[:, mt:mt + 1],
                    op0=ALU.mult, op1=ALU.add)
                if mt % 2 == 1:
                    yield
            # residual -> x2q (token-major)
            x2q = mqp.tile([P, 2, D], F32, tag="x2q", bufs=2)
            for j in range(2):
                tt = qq * 2 + j
                xr_t = mtp.tile([P, D], F32, tag="xr_t")
                nc.sync.dma_start(xr_t[:], xkv[tt * P:(tt + 1) * P, :])
                for m2 in range(KD // 2):
                    pst = tpair()
                    for h in range(2):
                        nc.tensor.transpose(
                            pst[:, h, :],
                            attnT[:, 2 * m2 + h, j * P:(j + 1) * P], ident[:])
                    nc.vector.tensor_tensor(
                        x2q[:, j, 2 * m2 * P:(2 * m2 + 2) * P],
                        pst.rearrange("p a m -> p (a m)"),
                        xr_t[:, 2 * m2 * P:(2 * m2 + 2) * P], ALU.add)
                yield
            # LN2 -> xn2 hi/lo (feature-major)
            xn2h = mqp.tile([P, KD, 256], FP8, tag="xn2h")
            xn2l = mqp.tile([P, KD, 256], FP8, tag="xn2l")
            for j in range(2):
                xn2_b = layernorm_tile(x2q[:, j, :], mtp, "l2",
                                       g2_rep if ln_affine else None,
                                       be2_rep if ln_affine else None)
                loc = slice(j * P, (j + 1) * P)
                for j2 in range(KD // 2):
                    pst = tpair()
                    for h in range(2):
                        nc.tensor.transpose(
                            pst[:, h, :],
                            xn2_b[:, (2 * j2 + h) * P:(2 * j2 + h + 1) * P],
                            ident[:])
                    nc.vector.tensor_copy(xn2h[:, 2 * j2:2 * j2 + 2, loc], pst)
                    nc.vector.tensor_tensor(
                        xn2l[:, 2 * j2:2 * j2 + 2, loc], pst,
                        xn2h[:, 2 * j2:2 * j2 + 2, loc], ALU.subtract)
                yield
            if DEBUG and qq == 0:
                nc.sync.dma_start(dbg["d_x2"], x2q[:])
            gens_state[qq] = (x2q, xn2h, xn2l)

        def gen_mlp_b(qq):
            # MLP fc1 (+gelu) with hi/lo split; gelus run as one contiguous
            # ACT block (2 act-table switches per quarter instead of ~28)
            _, xn2h, xn2l = gens_state[qq]
            # gelus read a per-quarter copy of b1 that depends on the LAST exp
            # of the following attention quarter: the act-table scheduler can
            # then never hoist a gelu into the exp stream (2 switches/quarter)
            e8last = gens_state.get(("e8", qq + 1))
            b1q = mqp.tile([P, FT], F32, tag="b1q", bufs=2)
            if e8last is not None:
                nc.vector.tensor_scalar_mul(b1q[:], e8last[:, 0, 0, 0:FT], 0.0)
                nc.vector.tensor_tensor(b1q[:], b1q[:], b1_t[:], ALU.add)
            else:
                nc.vector.tensor_copy(b1q[:], b1_t[:])
            h1h = mqp.tile([P, FT, 256], FP8, tag="h1h", bufs=2)
            h1l = mqp.tile([P, FT, 256], FP8, tag="h1l", bufs=2)
            w1_tiles = {}

            def w1_fetch(ft):
                t = mwp.tile([P, NC, 2, 2, P], FP8, tag="w1_t", bufs=6)
                nc.sync.dma_start(t[:], W1hl[ft])
                w1_tiles[ft] = t

            for ft in range(4):
                w1_fetch(ft)
            for ft in range(FT):
                if ft + 4 < FT:
                    w1_fetch(ft + 4)
                w1_t = w1_tiles.pop(ft)
                ps = aps.tile([P, 256], F32, tag="mpp", bufs=2)
                n3 = 3 * NC
                i = 0
                for hl, xt in ((0, xn2h), (1, xn2h), (0, xn2l)):
                    for c in range(NC):
                        nc.tensor.matmul(
                            ps[:], w1_t[:, c, :, hl, :], xt[:, 2 * c:2 * c + 2, :],
                            start=(i == 0), stop=(i == n3 - 1), perf_mode=DR)
                        i += 1
                h1b = mqp.tile([P, 256], BF16, tag="h1b", bufs=4)
                nc.scalar.activation(h1b[:], ps[:], AF.Gelu,
                                     bias=b1q[:, ft:ft + 1], scale=1.0 / WS)
                nc.gpsimd.tensor_copy(h1h[:, ft, :], h1b[:])
                nc.vector.tensor_tensor(
                    h1l[:, ft, :], h1b[:], h1h[:, ft, :], ALU.subtract)
                if ft % 8 == 7:
                    yield
            if DEBUG and qq == 0:
                nc.sync.dma_start(dbg["d_h1h"], h1h[:, 0:4, :])
                nc.sync.dma_start(dbg["d_h1l"], h1l[:, 0:4, :])
            for QT in (QpA, QpB):
                nc.vector.tensor_scalar_mul(
                    QT[:, :, NQ:NQ + 1], bcast8(h1h[:, FT - 1, 0:1]), 0.0)
            gens_state[qq] = (gens_state[qq][0], h1h, h1l)

        def gen_mlp_c(qq):
            x2q, h1h, h1l = gens_state[qq]
            tpb = tps.tile([P, 4, P], BF16, tag="tp")
            tslot = [0]

            def tsingle():
                s = tslot[0]
                tslot[0] = (s + 1) % 4
                return tpb[:, s, :]

            ob = mqp.tile([P, 2, D], F32, tag="ob")
            HC = FT // 4
            w2_tiles = {}

            def w2_fetch(mt):
                a = mwp.tile([P, HC, 2, 2, P], FP8, tag="w2a", bufs=3)
                nc.sync.dma_start(a[:], W2hl[mt, :, 0:HC])
                b = mwp.tile([P, HC, 2, 2, P], FP8, tag="w2b", bufs=3)
                nc.sync.dma_start(b[:], W2hl[mt, :, HC:2 * HC])
                w2_tiles[mt] = (a, b)

            w2_fetch(0)
            w2_fetch(1)
            for mt in range(KD):
                if mt + 2 < KD:
                    w2_fetch(mt + 2)
                w2a, w2b = w2_tiles.pop(mt)
                ps = aps.tile([P, 256], F32, tag="mpp", bufs=2)
                n3 = 3 * (FT // 2)
                i = 0
                for hl, ht in ((0, h1h), (1, h1h), (0, h1l)):
                    for c in range(FT // 2):
                        wt = w2a if c < HC else w2b
                        nc.tensor.matmul(
                            ps[:], wt[:, c % HC, :, hl, :],
                            ht[:, 2 * c:2 * c + 2, :],
                            start=(i == 0), stop=(i == n3 - 1), perf_mode=DR)
                        i += 1
                outT = mtp.tile([P, 256], BF16, tag="outT")
                nc.vector.tensor_scalar(
                    outT[:], ps[:], scalar1=1.0 / (2 * WS),
                    scalar2=b2_t[:, mt:mt + 1], op0=ALU.mult, op1=ALU.add)
                for j in range(2):
                    pst = tsingle()
                    nc.tensor.transpose(pst, outT[:, j * P:(j + 1) * P],
                                        ident[:])
                    nc.vector.tensor_tensor(
                        ob[:, j, mt * P:(mt + 1) * P], pst,
                        x2q[:, j, mt * P:(mt + 1) * P], ALU.add)
                yield
            for j in range(2):
                tt = qq * 2 + j
                nc.sync.dma_start(out[tt * P:(tt + 1) * P, :], ob[:, j, :])

        gens_state = {}

        def drain_all(g):
            if g is None:
                return
            for _ in g:
                pass

        def drain_n(g, n):
            if g is None:
                return
            for _ in range(n):
                try:
                    next(g)
                except StopIteration:
                    return

        A = {}
        B = {}
        C = {}
        for qq in range(4):
            a = gen_attn(qq)
            for i, _ in enumerate(a):
                if i < 4:
                    drain_n(A.get(qq - 1), 4)
                else:
                    drain_n(C.get(qq - 2), 3)
            drain_all(A.get(qq - 1))
            drain_all(C.get(qq - 2))
            drain_all(B.get(qq - 1))   # contiguous gelu block after the exps
            A[qq], B[qq], C[qq] = gen_mlp_a(qq), gen_mlp_b(qq), gen_mlp_c(qq)
        drain_all(A[3])
        drain_all(C[2])
        drain_all(B[3])
        drain_all(C[3])

        bf_es.close()
        kv_es.close()
        rt_es.close()

        es.close()

    nc.compile()
    return nc


def _q8(a, scale=1.0):
    return np.ascontiguousarray((a * scale)).astype(E4)


def _dr_tile(W, scale, m):
    """[Din, Dout] -> hi, lo tiles [Dout/m, 128, Din/256, 2, m] fp8."""
    Din, Dout = W.shape
    Ws = (W * scale).astype(np.float32)
    hi = Ws.astype(E4)
    lo = (Ws - hi.astype(np.float32)).astype(E4)

    def t(a):
        # f = c*256 + i*128 + p ; o = ot*m + j
        return np.ascontiguousarray(
            a.reshape(Din // 256, 2, P, Dout // m, m).transpose(3, 2, 0, 1, 4))
    return t(hi), t(lo)


def _dr_tile_hl(W, scale, m):
    """[Din, Dout] -> packed [Dout/m, 128, Din/256, 2, {hi,lo}, m] fp8."""
    hi, lo = _dr_tile(W, scale, m)
    return np.ascontiguousarray(np.stack([hi, lo], axis=4))


def kernel(**inputs):
    inputs = {k: np.ascontiguousarray(np.asarray(v), dtype=np.float32)
              for k, v in inputs.items()}
    ln_affine = not (
        np.all(inputs["ln1_g"] == 1.0) and np.all(inputs["ln1_b"] == 0.0)
        and np.all(inputs["ln2_g"] == 1.0) and np.all(inputs["ln2_b"] == 0.0))
    has_bv = not np.all(inputs["bv"] == 0.0)
    has_qkb = not (np.all(inputs["bq"] == 0.0) and np.all(inputs["bk"] == 0.0))
    key = ("nc", ln_affine, has_bv, has_qkb)
    if key not in _CACHE:
        _CACHE[key] = _build(ln_affine, has_bv, has_qkb)
    nc = _CACHE[key]

    x = inputs["x"]
    Wqh, _ = _dr_tile(inputs["Wq"], WS, P)
    Wkh, _ = _dr_tile(inputs["Wk"], WS, P)
    Wvh, _ = _dr_tile(inputs["Wv"], WS, 256)
    Woh, _ = _dr_tile(inputs["Wo"], WS, P)
    W1hl = _dr_tile_hl(inputs["W1"], WS, P)
    W2hl = _dr_tile_hl(inputs["W2"], 2 * WS, P)

    shared = {
        "Wq": Wqh, "Wk": Wkh, "Wv": Wvh, "Wo": Woh,
        "W1hl": W1hl, "W2hl": W2hl,
        "bq32": WS * inputs["bq"], "bk32": WS * inputs["bk"],
        "bv32": WS * inputs["bv"],
        "bo": inputs["bo"], "b1": inputs["b1"], "b2": inputs["b2"],
        "g1": inputs["ln1_g"], "be1": inputs["ln1_b"],
        "g2": inputs["ln2_g"], "be2": inputs["ln2_b"],
    }
    in_maps = []
    for c in range(8):
        b, half = c // 2, c % 2
        m = dict(shared)
        m["xkv"] = np.ascontiguousarray(
            np.concatenate([x[b, half * NQ:(half + 1) * NQ, :],
                            x[b, (1 - half) * NQ:(2 - half) * NQ, :]], axis=0))
        in_maps.append(m)

    trace = bool(int(os.environ.get("KERNEL_TRACE", "0")))
    kw = {}
    if trace:
        kw = dict(trace=True, tmpdir=os.environ.get("KERNEL_TRACE_DIR") or None)
    res = bass_utils.run_bass_kernel_spmd(nc, in_maps, core_ids=list(range(8)), **kw)
    _CACHE["last_results"] = res
    _CACHE["nc"] = nc
    _CACHE["last_in_maps"] = in_maps

    outa = np.empty((B, S, D), dtype=np.float32)
    for c in range(8):
        b, half = c // 2, c % 2
        outa[b, half * NQ:(half + 1) * NQ, :] = res.results[c]["out"]
    return outa
